# revision 23
# baseline (speedup 1.0000x reference)
"""Trainium2 Bass kernel for an 8-layer GCN (MemoryEfficientGNN).

Strategy (8 NeuronCores, SPMD single program):
  - Nodes sharded across cores: core k owns real nodes [12500k, 12500(k+1)),
    padded to 12544 rows (98 tiles/windows of 128).
  - GCN norm factorizes: out[v] = dinv[v] * sum_{(u,v)} dinv[u]*h[u], so the
    per-edge weights are one-hot.
  - Per layer: h' = dinv*(x_own @ W) on PE (bf16), AllGather h' (f8) into
    HBM; the rank strips concatenate to a [100352,128] row layout, which is
    upconverted f8->bf16 by one flat converting DMA per 25088-row chunk
    (per-chunk so gathers on chunk c start as soon as its convert lands).
    Then scatter-aggregate own dest windows: per 128-edge batch, dma_gather
    source rows and one-hot matmul P^T.T @ G accumulated in PSUM per
    128-dest window.
  - dma_gather takes int16 indices (<32768) but its base address is a full
    64-bit immediate, so each gather reads a row-offset slice of the single
    bf16 tensor; batches are grouped (window-block of 4, chunk) so each
    gather call reads one chunk while PSUM holds 4 window accumulators.
  - Post: each finished window is one ACT copy (dinv scale fused) into a
    [128,98,128] z-strip; self term (dinv^2*xW from the dense phase), bias,
    LayerNorm and ReLU/residual are whole-strip ops (big DVE tensor ops +
    per-tile bn_stats/bn_aggr), cutting ~1080 post instructions per layer
    to ~310.
  - Final: per-graph mean pool via one-hot(batch-id) matmuls, AllReduce of
    [128 graphs, 128+1] partials, replicated MLP + sigmoid.

The batch schedule is static and identical on all cores (SPMD); per-core
data (indices, offsets, x rows) differ.
"""

import sys

sys.path.insert(0, "/opt/trn_rl_repo")

import numpy as np
import ml_dtypes

N_NODES = 100000
N_EDGES = 1600000
HID = 128
FC_DIM = 64
NUM_GRAPHS = 128
EPS = 1e-5
NCORES = 8
OWN = N_NODES // NCORES          # 12500 real nodes per core
NTILES = (OWN + 127) // 128      # 98 windows per core
R = NTILES * 128                 # 12544 padded rows per core
GPAD = R * NCORES                # 100352 padded global rows
P = 128
NCHUNK = 4
CHUNK = GPAD // NCHUNK           # 25088 rows per gather chunk (int16-safe)
WB = 4                           # windows per PSUM block

BF = ml_dtypes.bfloat16

# fp32 constant-blob layout: one DMA covers every fp32 constant so any
# first-reader sees a single DMA semaphore lane (walrus caps per-inst waits).
_CB_LAYOUT = [
    ("W_in", P), ("W_mid", P), ("W_out", P), ("identity", P), ("iota_f32", P),
    ("b_in_bc", P), ("g_in_bc", P), ("be_in_bc", P),
    ("b_mid_bc", P), ("g_mid_bc", P), ("be_mid_bc", P),
    ("b_out_bc", P), ("g_out_bc", P), ("be_out_bc", P),
    ("b1_bc", FC_DIM), ("W1", FC_DIM), ("W2", 1),
    ("epsb", 1), ("ones_col", 1), ("b2_bc", 1),
    ("dinvs", NTILES), ("dinv2s", NTILES), ("batchb", NTILES),
]
CB_OFF = {}
_o = 0
for _n, _w in _CB_LAYOUT:
    CB_OFF[_n] = _o
    _o += _w
CB_COLS = _o


def _cb_slices():
    return [(n, CB_OFF[n], w) for n, w in _CB_LAYOUT if n != "W2"]


# ---------------------------------------------------------------- host prep
def build_schedule(edge_index):
    """Static batch schedule + per-core index/offset arrays."""
    # self-loops are NOT gathered: the diagonal term dinv[v]*h'[v] is added
    # from the local h' strip in the scatter post. deg still counts them.
    row = np.asarray(edge_index[0], np.int64)
    col = np.asarray(edge_index[1], np.int64)

    core = col // OWN
    loc = col - core * OWN
    win = loc // P                         # dest window 0..97
    off = loc - win * P                    # dest offset in window
    skk = row // OWN                       # source core
    srr = row % OWN
    stt = srr // P                         # source tile
    spp = srr % P                          # source offset in tile
    gsrc = skk * R + srr                   # padded global source row
    chk = skk // 2                         # chunk = source core pair
    # transposed h_full layout: chunk row = 128*(k%2)+p, col block = t
    lsrc = (P * (skk % 2) + spp) * NTILES + stt

    key = (core * NTILES + win) * NCHUNK + chk
    cnt = np.bincount(key, minlength=NCORES * NTILES * NCHUNK).reshape(
        NCORES, NTILES, NCHUNK)
    bwc = np.maximum(1, -(-cnt.max(axis=0) // P))      # [NTILES, NCHUNK]

    # slot order: window blocks of WB; per block chunk passes c=0..3
    slot_base = np.zeros((NTILES, NCHUNK), np.int64)
    win_of, chunk_of = [], []
    segs = []                              # (chunk, b0, b1) per gather call
    pos = 0
    for wb0 in range(0, NTILES, WB):
        wins = list(range(wb0, min(wb0 + WB, NTILES)))
        for c in range(NCHUNK):
            b0 = pos
            for w in wins:
                slot_base[w, c] = pos
                n = int(bwc[w, c])
                win_of += [w] * n
                chunk_of += [c] * n
                pos += n
            segs.append((c, b0, pos))
    NB = pos
    win_of = np.asarray(win_of)
    chunk_of = np.asarray(chunk_of)
    first = np.zeros(NB, bool)
    first[slot_base[:, 0]] = True
    last = np.zeros(NB, bool)
    last[slot_base[:, NCHUNK - 1] + bwc[:, NCHUNK - 1] - 1] = True

    # pad slots (offb=255) read a spread of rows rather than all hitting
    # row 0: an all-identical gather index pattern measured ~15% slower.
    spread = ((np.arange(P)[:, None] * NTILES + np.arange(NB)[None, :])
              % CHUNK).astype(np.int16)
    src16 = np.broadcast_to(spread, (NCORES, P, NB)).copy()
    srcg = np.zeros((NCORES, P, NB), np.int32)   # emulator only
    offb = np.full((NCORES, P, NB), 255.0, np.float32)

    order = np.argsort(key, kind="stable")
    key_s = key[order]
    cg = key_s // (NTILES * NCHUNK)
    wg = (key_s // NCHUNK) % NTILES
    chg = key_s % NCHUNK
    grp_start = np.concatenate([[0], np.cumsum(cnt.ravel())[:-1]])
    j = np.arange(key_s.size) - grp_start[key_s]
    b = slot_base[wg, chg] + j // P
    p = j % P
    src16[cg, p, b] = lsrc[order].astype(np.int16)
    srcg[cg, p, b] = gsrc[order].astype(np.int32)
    offb[cg, p, b] = off[order].astype(np.float32)

    deg = np.bincount(col, minlength=N_NODES).astype(np.float32) + 1.0
    dinv = 1.0 / np.sqrt(deg)

    maxsegb = max(b1 - b0 for _, b0, b1 in segs)
    return dict(NB=NB, win_of=win_of, chunk_of=chunk_of, first=first,
                last=last, segs=segs, maxsegb=maxsegb,
                src16=src16, srcg=srcg, offb=offb, dinv=dinv)


def build_core_inputs(inputs, sched):
    """Per-core named arrays (consumed by emulate + pack_device_maps)."""
    x = np.asarray(inputs["x"], np.float32)
    batch = np.asarray(inputs["batch"], np.int32)
    dinv = sched["dinv"]

    common = {
        "W_in": np.asarray(inputs["W_in"], np.float32),
        "W_mid": np.asarray(inputs["W_mid"], np.float32),
        "W_out": np.asarray(inputs["W_out"], np.float32),
        "W1": np.asarray(inputs["W1"], np.float32),
        "W2": np.asarray(inputs["W2"], np.float32),
        "identity": np.eye(P, dtype=np.float32),
        "iota_bf": np.tile(np.arange(P, dtype=np.float32), (P, 1)).astype(BF),
        "iota_f32": np.tile(np.arange(P, dtype=np.float32), (P, 1)),
        "ones_col": np.ones((P, 1), np.float32),
        "epsb": np.full((P, 1), EPS, np.float32),
        "b2_bc": np.full((P, 1), float(np.asarray(inputs["b2"])[0]), np.float32),
        "b1_bc": np.tile(np.asarray(inputs["b1"], np.float32), (P, 1)),
    }
    for nm in ("in", "mid", "out"):
        common[f"b_{nm}_bc"] = np.tile(np.asarray(inputs[f"b_{nm}"], np.float32),
                                       (P, 1))
        common[f"g_{nm}_bc"] = np.tile(np.asarray(inputs[f"g_{nm}"], np.float32),
                                       (P, 1))
        common[f"be_{nm}_bc"] = np.tile(np.asarray(inputs[f"be_{nm}"], np.float32),
                                        (P, 1))

    maps = []
    for k in range(NCORES):
        lo, hi = k * OWN, (k + 1) * OWN
        xs = np.zeros((R, HID), np.float32)
        xs[:OWN] = x[lo:hi]
        tmp = np.zeros(R, np.float32)
        tmp[:OWN] = dinv[lo:hi]
        dv = np.ascontiguousarray(tmp.reshape(NTILES, P).T)
        dv2 = np.ascontiguousarray((tmp * tmp).reshape(NTILES, P).T)
        tmp = np.full(R, 255.0, np.float32)
        tmp[:OWN] = batch[lo:hi].astype(np.float32)
        bb = np.ascontiguousarray(tmp.reshape(NTILES, P).T)
        m = dict(common)
        m.update({
            "xs": xs,
            "src16": sched["src16"][k],
            "srcg": sched["srcg"][k],
            "offb": sched["offb"][k].astype(BF),
            "dinvs": dv,
            "dinv2s": dv2,
            "batchb": bb,
        })
        maps.append(m)
    return maps


def pack_device_maps(maps, sched):
    """Pack named arrays into device in_maps."""
    NB = sched["NB"]
    dev = []
    for m in maps:
        cb = np.zeros((P, CB_COLS), np.float32)
        for n, w in _CB_LAYOUT:
            o = CB_OFF[n]
            a = np.asarray(m[n], np.float32)
            if n == "W2":
                cb[:FC_DIM, o:o + 1] = a
            else:
                cb[:, o:o + w] = a
        # idx16: per segment, wrap idxs (j = batch*128 + p) into 16
        # partitions, col j//16; replicate to all 8 gpsimd core groups.
        idx16 = np.zeros((P, NB * 8), np.int16)
        for c, b0, b1 in sched["segs"]:
            jarr = m["src16"][:, b0:b1].T.reshape(-1)        # [n] batch-major
            blk = jarr.reshape(-1, 16).T                     # [16, n/16]
            idx16[:, b0 * 8:b1 * 8] = np.tile(blk, (8, 1))
        dev.append({
            "xs": m["xs"].astype(BF),
            "srcidx": idx16,
            "offb": m["offb"],
            "iota_bf": m["iota_bf"],
            "cblob": cb,
        })
    return dev


# ------------------------------------------------------------ numpy emulator
def emulate(inputs, sched, maps):
    """Emulation of the device numerics (bf16 activations/weights)."""
    NB = sched["NB"]
    win_of = sched["win_of"]
    layers = (["in"] + ["mid"] * 6 + ["out"])

    xs = [m["xs"].astype(BF) for m in maps]
    for li, nm in enumerate(layers):
        W = maps[0][f"W_{nm}"].astype(BF).astype(np.float32)
        b = maps[0][f"b_{nm}_bc"][0]
        g = maps[0][f"g_{nm}_bc"][0]
        be = maps[0][f"be_{nm}_bc"][0]
        import ml_dtypes as _md
        F8 = _md.float8_e4m3
        hp = []
        hp2s = []
        for k in range(NCORES):
            h = xs[k].astype(np.float32) @ W
            dv = maps[k]["dinvs"].T.reshape(-1, 1)
            dv2 = maps[k]["dinv2s"].T.reshape(-1, 1)
            hp.append((h * dv).astype(F8))
            hp2s.append((hp[-1].astype(np.float32) * dv).astype(F8))
        h_full = np.concatenate(hp, 0).astype(BF).astype(np.float32)
        nxt = []
        for k in range(NCORES):
            m = maps[k]
            out = np.zeros((R, HID), np.float32)
            for bidx in range(NB):
                w = win_of[bidx]
                G = h_full[m["srcg"][:, bidx]]
                offv = m["offb"][:, bidx].astype(np.float32)
                Pm = (offv[:, None] == np.arange(P)).astype(np.float32)
                out[w * P:(w + 1) * P] += Pm.T @ G
            dv = m["dinvs"].T.reshape(-1, 1)
            z = (out * dv + hp2s[k].astype(np.float32) + b).astype(BF)
            z = z.astype(np.float32)
            mu = z.mean(1, keepdims=True)
            var = ((z - mu) ** 2).mean(1, keepdims=True)
            y = ((z - mu) / np.sqrt(var + EPS) * g + be).astype(BF)
            y = np.maximum(y.astype(np.float32), 0.0)
            if nm == "mid" and li % 2 == 0:
                y = (y.astype(BF).astype(np.float32) +
                     xs[k].astype(np.float32)).astype(BF)
            else:
                y = y.astype(BF)
            nxt.append(y)
        xs = nxt

    sums = np.zeros((NUM_GRAPHS, HID + 1), np.float32)
    for k in range(NCORES):
        bb = maps[k]["batchb"].astype(np.float32).T.reshape(-1)
        valid = bb < NUM_GRAPHS
        idx = bb[valid].astype(np.int64)
        np.add.at(sums[:, :HID], idx, xs[k].astype(np.float32)[valid])
        np.add.at(sums[:, HID], idx, 1.0)
    pooled = sums[:, :HID] / np.maximum(sums[:, HID:], 1.0)
    z = np.maximum(pooled @ maps[0]["W1"] + maps[0]["b1_bc"][0], 0.0)
    o = z @ maps[0]["W2"] + maps[0]["b2_bc"][0, 0]
    return 1.0 / (1.0 + np.exp(-o))


# ------------------------------------------------------------- bass program
def build_nc(sched, taps=()):
    import concourse.bass as bass
    import concourse.bacc as bacc
    import concourse.tile as tile
    from concourse import mybir

    NB = sched["NB"]
    win_of, first, last = sched["win_of"], sched["first"], sched["last"]
    segs, maxsegb = sched["segs"], sched["maxsegb"]
    f32 = mybir.dt.float32
    bf16 = mybir.dt.bfloat16
    f8 = mybir.dt.float8e4
    i16 = mybir.dt.int16
    AF = mybir.ActivationFunctionType
    OP = mybir.AluOpType

    nc = bacc.Bacc("TRN2", target_bir_lowering=False, debug=False,
                   num_devices=NCORES, num_swdge_queues=4)

    xs_d = nc.dram_tensor("xs", [R, HID], bf16, kind="ExternalInput")
    src_d = nc.dram_tensor("srcidx", [P, NB * 8], i16, kind="ExternalInput")
    off_d = nc.dram_tensor("offb", [P, NB], bf16, kind="ExternalInput")
    cb_d = nc.dram_tensor("cblob", [P, CB_COLS], f32, kind="ExternalInput")
    iobf_d = nc.dram_tensor("iota_bf", [P, P], bf16, kind="ExternalInput")
    out_d = nc.dram_tensor("out", [NUM_GRAPHS, 1], f32, kind="ExternalOutput")
    tap_d = {}
    for tn, shp, dt in [("hfull0", [GPAD, HID], mybir.dt.float8e4),
                        ("x1", [P, NTILES, P], bf16),
                        ("G0", [P, sched["maxsegb"] * P], bf16),
                        ("P0", [P, sched["maxsegb"] * P], bf16)]:
        if tn in taps:
            tap_d[tn] = nc.dram_tensor(tn, shp, dt, kind="ExternalOutput")

    layers = (["in"] + ["mid"] * 6 + ["out"])

    with tile.TileContext(nc) as tc:
        with (
            tc.tile_pool(name="singles", bufs=1) as singles,
            tc.tile_pool(name="xab", bufs=1) as xab,
            tc.tile_pool(name="sbA", bufs=3) as sbA,
            tc.tile_pool(name="hppool", bufs=1) as hppool,
            tc.tile_pool(name="zstrips", bufs=1) as zstrips,
            tc.tile_pool(name="statp", bufs=1) as statp,
            tc.tile_pool(name="ipool", bufs=1) as ipool,
            tc.tile_pool(name="gpool", bufs=4) as gpool,
            tc.tile_pool(name="ppool", bufs=3) as ppool,
            tc.tile_pool(name="zpool", bufs=3) as zpool,
            tc.tile_pool(name="spool", bufs=4) as spool,
            tc.tile_pool(name="psA", bufs=3, space="PSUM") as psA,
            tc.tile_pool(name="psW", bufs=4, space="PSUM") as psW,
            tc.tile_pool(name="psP", bufs=1, space="PSUM") as psP,
            tc.tile_pool(name="dram", bufs=2, space="DRAM") as dram,
            tc.tile_pool(name="dram1", bufs=1, space="DRAM") as dram1,
        ):
            # ---- constants
            cblob = singles.tile([P, CB_COLS], f32, name="cblob")
            nc.sync.dma_start(cblob[:], cb_d[:, :])
            consts = {}
            for name, o, w in _cb_slices():
                consts[name] = cblob[:, o:o + w]
            consts["W2"] = cblob[0:FC_DIM, CB_OFF["W2"]:CB_OFF["W2"] + 1]
            iota_sb = singles.tile([P, P], bf16, name="iota_sb")
            nc.sync.dma_start(iota_sb[:], iobf_d[:, :])
            off_sb = singles.tile([P, NB], bf16, name="off_sb")
            nc.sync.dma_start(off_sb[:], off_d[:, :])
            dinv_sb = consts["dinvs"]
            dinv2_sb = consts["dinv2s"]
            bat_sb = consts["batchb"]

            # bf16 copies of matmul weights / identity
            wbf = {}
            for nm in ("in", "mid", "out"):
                t = singles.tile([P, P], bf16, name=f"Wbf_{nm}")
                nc.vector.tensor_copy(t[:], consts[f"W_{nm}"])
                wbf[nm] = t
            ident_bf = singles.tile([P, P], bf16, name="ident_bf")
            nc.vector.tensor_copy(ident_bf[:], consts["identity"])

            xa = xab.tile([P, NTILES, P], bf16, name="xa")
            xb = xab.tile([P, NTILES, P], bf16, name="xb")
            nc.sync.dma_start(
                xa[:], xs_d.rearrange("(t p) f -> p t f", p=P))

            onesbf = singles.tile([P, 1], bf16, name="onesbf")
            nc.vector.tensor_copy(onesbf[:], consts["ones_col"][:])

            def dense_phase(xcur, nm):
                """h' = dinv*(x @ W) -> SBUF strip -> one [128,12544] store.

                The self-loop term dinv^2 * (x @ W) is derived from this
                strip by the strip post (one big DVE multiply)."""
                h_own = dram.tile([P, NTILES * P], f8, name="h_own")
                hp_strip = hppool.tile([P, NTILES, P], f8, name="hp_strip")
                W = wbf[nm]
                for t in range(NTILES):
                    xT_ps = psA.tile([P, P], bf16, name="xT_ps", tag="psa")
                    nc.tensor.transpose(xT_ps[:], xcur[:, t, :], ident_bf[:])
                    xT_sb = sbA.tile([P, P], bf16, name="xT_sb")
                    nc.vector.tensor_copy(xT_sb[:], xT_ps[:])
                    h_ps = psA.tile([P, P], f32, name="h_ps", tag="psa")
                    nc.tensor.matmul(h_ps[:], lhsT=xT_sb[:], rhs=W[:],
                                     start=True, stop=True)
                    nc.scalar.activation(hp_strip[:, t, :], h_ps[:], AF.Copy,
                                         scale=dinv_sb[:, t:t + 1])
                nc.sync.dma_start(h_own[:, :], hp_strip[:])
                return h_own, hp_strip

            def scatter_phase(h_full, Tbf, hp_strip, xcur, xnxt, nm,
                              residual, idx_strip, zstrip, tap=False,
                              half_cb=None):
                accs = {}
                HALF = (NTILES // 2 // WB) * WB   # window-block boundary
                done = [0]
                for si, (c, b0, b1) in enumerate(segs):
                    nb = b1 - b0
                    isl = idx_strip[:, b0 * 8:b1 * 8]
                    # gather base address is a 64-bit immediate from the AP,
                    # so slicing the big bf16 tensor at the chunk offset is
                    # fine; only the int16 index must stay < 32768.
                    src_ap = Tbf[c * CHUNK:(c + 1) * CHUNK, :]

                    Gt = gpool.tile([P, maxsegb * P], bf16, name="Gt")
                    nc.gpsimd.dma_gather(
                        out_ap=Gt[:, :nb * P].rearrange(
                            "p (b f) -> p b f", b=nb),
                        in_ap=src_ap,
                        idxs_ap=isl,
                        num_idxs=nb * P,
                        num_idxs_reg=nb * P,
                        elem_size=P,
                        single_packet=False,
                        queue_num=si % 4,
                    )
                    if tap and si == 0 and "G0" in tap_d:
                        nc.sync.dma_start(tap_d["G0"][:, :nb * P],
                                          Gt[:, :nb * P])
                    Pt = ppool.tile([P, maxsegb * P], bf16, name="Pt")
                    o = off_sb[:, b0:b1]
                    off_b = bass.AP(tensor=o.tensor, offset=o.offset,
                                    ap=list(o.ap) + [[0, P]])
                    i0 = iota_sb[:, :]
                    iota_b = bass.AP(tensor=i0.tensor, offset=i0.offset,
                                     ap=[i0.ap[0], [0, nb], [1, P]])
                    nc.vector.tensor_tensor(
                        out=Pt[:, :nb * P].rearrange("p (b f) -> p b f", b=nb),
                        in0=off_b, in1=iota_b, op=OP.is_equal)
                    if tap and si == 0 and "P0" in tap_d:
                        nc.sync.dma_start(tap_d["P0"][:, :nb * P],
                                          Pt[:, :nb * P])
                    for bi in range(b0, b1):
                        w = int(win_of[bi])
                        s = bi - b0
                        if first[bi]:
                            accs[w] = psW.tile([P, P], f32, name="acc")
                        acc = accs[w]
                        nc.tensor.matmul(
                            acc[:], lhsT=Pt[:, s * P:(s + 1) * P],
                            rhs=Gt[:, s * P:(s + 1) * P],
                            start=bool(first[bi]), stop=bool(last[bi]))
                        if last[bi]:
                            del accs[w]
                            # z[w] = acc * dinv  (self/bias/LN done on the
                            # whole strip afterwards with big ops)
                            nc.scalar.activation(zstrip[:, w, :], acc[:],
                                                 AF.Copy,
                                                 scale=dinv_sb[:, w:w + 1])
                            done[0] += 1
                            if done[0] == HALF and half_cb is not None:
                                half_cb(0, HALF)
                if half_cb is not None:
                    half_cb(HALF, NTILES)

            def strip_post(zstrip, hp_strip, xcur, xnxt, nm, residual,
                           t0, t1):
                """Self-term + bias + LayerNorm + ReLU on tiles [t0, t1)."""
                nt = t1 - t0
                b_bc = consts[f"b_{nm}_bc"]
                g_bc = consts[f"g_{nm}_bc"]
                be_bc = consts[f"be_{nm}_bc"]

                def bc_t(const_pp):
                    # [P, P] feature constant -> [p, {0,nt}, f]
                    a = const_pp[:, :]
                    return bass.AP(tensor=a.tensor, offset=a.offset,
                                   ap=[a.ap[0], [0, nt], a.ap[1]])

                def bc_f(strip2d):
                    # [P, NTILES]-shaped AP -> [p, t, {0,P}]
                    a = strip2d
                    return bass.AP(tensor=a.tensor, offset=a.offset,
                                   ap=[a.ap[0], a.ap[1], [0, P]])

                # self term: hp * dinv (hp already carries one dinv factor)
                zs = zstrip[:, t0:t1, :]
                hp2t = hppool.tile([P, NTILES, P], f8, name="hp2t",
                                   tag="hp2t")
                dv = dinv_sb[:, t0:t1]
                nc.vector.tensor_tensor(
                    out=hp2t[:, t0:t1, :], in0=hp_strip[:, t0:t1, :],
                    in1=bass.AP(tensor=dv.tensor, offset=dv.offset,
                                ap=list(dv.ap) + [[0, P]]),
                    op=OP.mult)
                nc.vector.tensor_add(zs, zs, hp2t[:, t0:t1, :])
                nc.vector.tensor_tensor(out=zs, in0=zs,
                                        in1=bc_t(b_bc), op=OP.add)
                stats6 = statp.tile([P, NTILES, 6], f32, name="stats6",
                                    tag="st6")
                mv = statp.tile([P, NTILES, 2], f32, name="mvs", tag="mvs")
                for t in range(t0, t1):
                    nc.vector.bn_stats(stats6[:, t, :], zstrip[:, t, :])
                    nc.vector.bn_aggr(mv[:, t, :], stats6[:, t, :])
                mu2 = mv[:, t0:t1, 0:1]
                var2 = mv[:, t0:t1, 1:2]
                sd = statp.tile([P, NTILES], f32, name="sds", tag="sds")
                nc.scalar.activation(
                    sd[:, t0:t1],
                    bass.AP(tensor=var2.tensor, offset=var2.offset,
                            ap=[var2.ap[0], var2.ap[1]]),
                    AF.Sqrt, bias=consts["epsb"][:])
                rstd = statp.tile([P, NTILES], f32, name="rstds", tag="rst")
                nc.vector.reciprocal(rstd[:, t0:t1], sd[:, t0:t1])
                nc.vector.tensor_tensor(
                    out=zs, in0=zs,
                    in1=bass.AP(tensor=mu2.tensor, offset=mu2.offset,
                                ap=[mu2.ap[0], mu2.ap[1], [0, P]]),
                    op=OP.subtract)
                rs = rstd[:, t0:t1]
                nc.vector.tensor_tensor(out=zs, in0=zs,
                                        in1=bc_f(rs), op=OP.mult)
                nc.vector.tensor_tensor(out=zs, in0=zs,
                                        in1=bc_t(g_bc), op=OP.mult)
                nc.vector.tensor_tensor(out=zs, in0=zs,
                                        in1=bc_t(be_bc), op=OP.add)
                if residual:
                    nc.vector.tensor_scalar_max(out=zs, in0=zs, scalar1=0.0)
                    nc.vector.tensor_add(xnxt[:, t0:t1, :], zs,
                                         xcur[:, t0:t1, :])
                else:
                    nc.scalar.activation(xnxt[:, t0:t1, :], zs, AF.Relu)

            cur, nxt = xa, xb
            idx_strip = ipool.tile([P, NB * 8], i16, name="idx_strip")
            nc.sync.dma_start(idx_strip[:], src_d[:, :])
            for li, nm in enumerate(layers):
                h_own, hp_strip = dense_phase(cur, nm)
                h_full = dram.tile([P * NCORES, NTILES * P], f8,
                                   addr_space="Shared", name="h_full")
                nc.gpsimd.collective_compute(
                    "AllGather", OP.bypass,
                    replica_groups=[list(range(NCORES))],
                    ins=[h_own[:, :].opt()], outs=[h_full[:, :].opt()])
                # rank strips in [p, t, f] order concatenate to exactly the
                # [100352, 128] chunk-stacked row layout; the f8 -> bf16
                # upconvert is a flat converting DMA per chunk (per-chunk so
                # gathers on chunk c start as soon as its convert lands).
                Tbf_t = dram.tile([GPAD, HID], bf16, name="Tbf")
                for c in range(NCHUNK):
                    nc.gpsimd.dma_start(
                        Tbf_t[c * CHUNK:(c + 1) * CHUNK, :].rearrange(
                            "(q r) f -> q (r f)", q=2 * P),
                        h_full[c * 2 * P:(c + 1) * 2 * P, :])
                Tbf = Tbf_t[:, :]
                residual = (nm == "mid" and li % 2 == 0)
                zstrip = zstrips.tile([P, NTILES, P], bf16, name="zstrip",
                                      tag="zs")
                scatter_phase(h_full, Tbf, hp_strip, cur, nxt, nm,
                              residual, idx_strip, zstrip, tap=(li == 0),
                              half_cb=lambda t0, t1: strip_post(
                                  zstrip, hp_strip, cur, nxt, nm, residual,
                                  t0, t1))
                if li == 0 and "x1" in tap_d:
                    nc.sync.dma_start(tap_d["x1"][:, :, :], nxt[:])
                cur, nxt = nxt, cur

            # ---- pooling
            # one-hot strip built here so its buffer (zstrips pool) is free
            # during the layers: Bstrip[p, t, g] = (batch[p,t] == g)
            Bstrip = zstrips.tile([P, NTILES, P], bf16, name="Bstrip",
                                  tag="zs")
            _bat = bat_sb[:, :]
            bat_b = bass.AP(tensor=_bat.tensor, offset=_bat.offset,
                            ap=list(_bat.ap) + [[0, P]])
            _io = consts["iota_f32"][:, :]
            iota_rep = bass.AP(tensor=_io.tensor, offset=_io.offset,
                               ap=[_io.ap[0], [0, NTILES], [1, P]])
            nc.vector.tensor_tensor(out=Bstrip[:], in0=bat_b, in1=iota_rep,
                                    op=OP.is_equal)
            pool_ps = psP.tile([P, HID + 1], f32, name="pool_ps")
            for t in range(NTILES):
                nc.tensor.matmul(pool_ps[:, :HID], lhsT=Bstrip[:, t, :],
                                 rhs=cur[:, t, :],
                                 start=(t == 0), stop=(t == NTILES - 1),
                                 skip_group_check=True)
                nc.tensor.matmul(pool_ps[:, HID:HID + 1], lhsT=Bstrip[:, t, :],
                                 rhs=onesbf[:],
                                 start=(t == 0), stop=(t == NTILES - 1),
                                 skip_group_check=True)
            pool_sb = zpool.tile([P, HID + 1], f32, name="pool_sb")
            nc.vector.tensor_copy(pool_sb[:], pool_ps[:])
            pool_in = dram1.tile([P, HID + 1], f32, name="pool_in")
            pool_out = dram1.tile([P, HID + 1], f32, addr_space="Shared",
                                  name="pool_out")
            nc.sync.dma_start(pool_in[:, :], pool_sb[:])
            nc.gpsimd.collective_compute(
                "AllReduce", OP.add, replica_groups=[list(range(NCORES))],
                ins=[pool_in[:, :].opt()], outs=[pool_out[:, :].opt()])
            pooled = zpool.tile([P, HID + 1], f32, name="pooled")
            nc.sync.dma_start(pooled[:], pool_out[:, :])

            cnt = spool.tile([P, 1], f32, name="cnt")
            nc.vector.tensor_copy(cnt[:], pooled[:, HID:HID + 1])
            nc.vector.tensor_scalar_max(out=cnt[:], in0=cnt[:], scalar1=1.0)
            crec = spool.tile([P, 1], f32, name="crec")
            nc.vector.reciprocal(crec[:], cnt[:])
            pm = zpool.tile([P, HID], f32, name="pm")
            nc.vector.tensor_scalar_mul(out=pm[:], in0=pooled[:, :HID],
                                        scalar1=crec[:])
            pmT_ps = psA.tile([P, P], f32, name="pmT_ps", tag="psa")
            nc.tensor.transpose(pmT_ps[:], pm[:], consts["identity"])
            pmT = sbA.tile([P, P], f32, name="pmT")
            nc.vector.tensor_copy(pmT[:], pmT_ps[:])
            z1_ps = psA.tile([P, FC_DIM], f32, name="z1_ps", tag="psa")
            nc.tensor.matmul(z1_ps[:], lhsT=pmT[:], rhs=consts["W1"],
                             start=True, stop=True)
            z1 = zpool.tile([P, FC_DIM], f32, name="z1")
            nc.vector.tensor_add(z1[:], z1_ps[:], consts["b1_bc"])
            nc.scalar.activation(z1[:], z1[:], AF.Relu)
            z1T_ps = psA.tile([FC_DIM, P], f32, name="z1T_ps", tag="psa")
            nc.tensor.transpose(z1T_ps[:], z1[:], consts["identity"])
            z1T = sbA.tile([FC_DIM, P], f32, name="z1T")
            nc.vector.tensor_copy(z1T[:], z1T_ps[:])
            o_ps = psA.tile([P, 1], f32, name="o_ps", tag="psa")
            nc.tensor.matmul(o_ps[:], lhsT=z1T[:], rhs=consts["W2"],
                             start=True, stop=True)
            o_sb = spool.tile([P, 1], f32, name="o_sb")
            nc.scalar.activation(o_sb[:], o_ps[:], AF.Sigmoid,
                                 bias=consts["b2_bc"][:])
            nc.sync.dma_start(out_d[:, :], o_sb[:])

    nc.compile()
    return nc


# ----------------------------------------------------------------- entry
_CACHE = {}


def kernel(**inputs):
    from concourse import bass_utils

    edge_index = np.asarray(inputs["edge_index"])
    sched = build_schedule(edge_index)
    maps = build_core_inputs(inputs, sched)

    key = sched["NB"]
    if key not in _CACHE:
        _CACHE[key] = build_nc(sched)
    nc = _CACHE[key]

    res = bass_utils.run_bass_kernel_spmd(
        nc, pack_device_maps(maps, sched), core_ids=list(range(NCORES)))
    return np.asarray(res.results[0]["out"], np.float32)



# revision 25
# speedup vs baseline: 1.0280x; 1.0280x over previous
"""Trainium2 Bass kernel for an 8-layer GCN (MemoryEfficientGNN).

Strategy (8 NeuronCores, SPMD single program):
  - Nodes sharded across cores: core k owns real nodes [12500k, 12500(k+1)),
    padded to 12544 rows (98 tiles/windows of 128).
  - GCN norm factorizes: out[v] = dinv[v] * sum_{(u,v)} dinv[u]*h[u], so the
    per-edge weights are one-hot.
  - Per layer: h' = dinv*(x_own @ W) on PE (bf16), AllGather h' (f8) into
    HBM; the rank strips concatenate to a [100352,128] row layout, which is
    upconverted f8->bf16 by one flat converting DMA per 25088-row chunk
    (per-chunk so gathers on chunk c start as soon as its convert lands).
    Then scatter-aggregate own dest windows: per 128-edge batch, dma_gather
    source rows and one-hot matmul P^T.T @ G accumulated in PSUM per
    128-dest window.
  - dma_gather takes int16 indices (<32768) but its base address is a full
    64-bit immediate, so each gather reads a row-offset slice of the single
    bf16 tensor; batches are grouped (window-block of 4, chunk) so each
    gather call reads one chunk while PSUM holds 4 window accumulators.
  - Post: each finished window is one ACT copy (dinv scale fused) into a
    [128,98,128] z-strip; self term (dinv^2*xW from the dense phase), bias,
    LayerNorm and ReLU/residual are whole-strip ops (big DVE tensor ops +
    per-tile bn_stats/bn_aggr), cutting ~1080 post instructions per layer
    to ~310.
  - Final: per-graph mean pool via one-hot(batch-id) matmuls, AllReduce of
    [128 graphs, 128+1] partials, replicated MLP + sigmoid.

The batch schedule is static and identical on all cores (SPMD); per-core
data (indices, offsets, x rows) differ.
"""

import sys

sys.path.insert(0, "/opt/trn_rl_repo")

import numpy as np
import ml_dtypes

N_NODES = 100000
N_EDGES = 1600000
HID = 128
FC_DIM = 64
NUM_GRAPHS = 128
EPS = 1e-5
NCORES = 8
OWN = N_NODES // NCORES          # 12500 real nodes per core
NTILES = (OWN + 127) // 128      # 98 windows per core
R = NTILES * 128                 # 12544 padded rows per core
GPAD = R * NCORES                # 100352 padded global rows
P = 128
NCHUNK = 4
CHUNK = GPAD // NCHUNK           # 25088 rows per gather chunk (int16-safe)
WB = 4                           # windows per PSUM block

BF = ml_dtypes.bfloat16

# fp32 constant-blob layout: one DMA covers every fp32 constant so any
# first-reader sees a single DMA semaphore lane (walrus caps per-inst waits).
_CB_LAYOUT = [
    ("W_in", P), ("W_mid", P), ("W_out", P), ("identity", P), ("iota_f32", P),
    ("b_in_bc", P), ("g_in_bc", P), ("be_in_bc", P),
    ("b_mid_bc", P), ("g_mid_bc", P), ("be_mid_bc", P),
    ("b_out_bc", P), ("g_out_bc", P), ("be_out_bc", P),
    ("b1_bc", FC_DIM), ("W1", FC_DIM), ("W2", 1),
    ("epsb", 1), ("ones_col", 1), ("b2_bc", 1),
    ("dinvs", NTILES), ("dinv2s", NTILES), ("batchb", NTILES),
]
CB_OFF = {}
_o = 0
for _n, _w in _CB_LAYOUT:
    CB_OFF[_n] = _o
    _o += _w
CB_COLS = _o


def _cb_slices():
    return [(n, CB_OFF[n], w) for n, w in _CB_LAYOUT if n != "W2"]


# ---------------------------------------------------------------- host prep
def build_schedule(edge_index):
    """Static batch schedule + per-core index/offset arrays."""
    # self-loops are NOT gathered: the diagonal term dinv[v]*h'[v] is added
    # from the local h' strip in the scatter post. deg still counts them.
    row = np.asarray(edge_index[0], np.int64)
    col = np.asarray(edge_index[1], np.int64)

    core = col // OWN
    loc = col - core * OWN
    skk = row // OWN                       # source core
    chk = skk // 2                         # chunk = source core pair

    # --- degree-aware within-core window balancing ---------------------
    # Pooling is permutation-invariant, so nodes may be placed in any
    # window of their core. The per-(window,chunk) mean in-degree is
    # 510.2, just under 4*128: a greedy min-max packer drives nearly all
    # cells to <=512, cutting batches/window/chunk from ~5 to ~4.
    # All cores must overflow the SAME cells (bwc is a max over cores), so
    # a shared capacity map routes spikes into 3 designated 640-slot
    # windows; everything else is packed to <=512 (4 batches).
    cap = np.full((NTILES, NCHUNK), 4 * P, np.float64)
    cap[NTILES - 3:, :] = 5 * P
    lslot = np.zeros((NCORES, OWN), np.int64)
    for k in range(NCORES):
        m = core == k
        D = np.zeros((OWN, NCHUNK), np.int64)
        np.add.at(D, (loc[m], chk[m]), 1)
        order = np.argsort(-D.sum(1), kind="stable")
        L = np.zeros((NTILES, NCHUNK), np.float64)
        nfill = np.zeros(NTILES, np.int64)
        slot_of = np.zeros(OWN, np.int64)
        members = [[] for _ in range(NTILES)]
        for i in order:
            proj = L + D[i]
            score = (np.maximum(proj - cap, 0.0).sum(1) * 1e9
                     + (proj * proj).sum(1))
            score[nfill >= P] = 1e18
            w = int(np.argmin(score))
            L[w] += D[i]
            slot_of[i] = w * P + nfill[w]
            nfill[w] += 1
            members[w].append(i)
        # swap-repair residual over-cap cells
        for _ in range(4):
            over = np.argwhere(L > cap)
            if len(over) == 0:
                break
            for w, c in over:
                need = L[w, c] - cap[w, c]
                for a in sorted(members[w], key=lambda i: -D[i, c]):
                    if need <= 0:
                        break
                    if D[a, c] == 0:
                        break
                    placed = False
                    for w2 in np.argsort(L[:, c] - cap[:, c])[:24]:
                        if w2 == w or not members[w2]:
                            continue
                        b = min(members[w2], key=lambda i: D[i, c])
                        delta = (D[a] - D[b]).astype(np.float64)
                        if delta[c] <= 0:
                            continue
                        if np.all(L[w2] + delta <= cap[w2]):
                            L[w] -= delta
                            L[w2] += delta
                            members[w].remove(a)
                            members[w2].remove(b)
                            members[w].append(b)
                            members[w2].append(a)
                            slot_of[a], slot_of[b] = slot_of[b], slot_of[a]
                            need -= delta[c]
                            placed = True
                            break
                    if not placed:
                        break
        lslot[k] = slot_of

    l_dst = lslot[core, loc]
    win = l_dst // P                       # dest window 0..97
    off = l_dst - win * P                  # dest offset in window
    l_src = lslot[skk, row % OWN]
    stt = l_src // P                       # source tile
    spp = l_src % P                        # source offset in tile
    gsrc = skk * R + l_src                 # padded global source row
    # transposed h_full layout: chunk row = 128*(k%2)+p, col block = t
    lsrc = (P * (skk % 2) + spp) * NTILES + stt

    key = (core * NTILES + win) * NCHUNK + chk
    cnt = np.bincount(key, minlength=NCORES * NTILES * NCHUNK).reshape(
        NCORES, NTILES, NCHUNK)
    bwc = np.maximum(1, -(-cnt.max(axis=0) // P))      # [NTILES, NCHUNK]

    # slot order: window blocks of WB; per block chunk passes c=0..3
    slot_base = np.zeros((NTILES, NCHUNK), np.int64)
    win_of, chunk_of = [], []
    segs = []                              # (chunk, b0, b1) per gather call
    pos = 0
    for wb0 in range(0, NTILES, WB):
        wins = list(range(wb0, min(wb0 + WB, NTILES)))
        for c in range(NCHUNK):
            b0 = pos
            for w in wins:
                slot_base[w, c] = pos
                n = int(bwc[w, c])
                win_of += [w] * n
                chunk_of += [c] * n
                pos += n
            segs.append((c, b0, pos))
    NB = pos
    win_of = np.asarray(win_of)
    chunk_of = np.asarray(chunk_of)
    first = np.zeros(NB, bool)
    first[slot_base[:, 0]] = True
    last = np.zeros(NB, bool)
    last[slot_base[:, NCHUNK - 1] + bwc[:, NCHUNK - 1] - 1] = True

    # pad slots (offb=255) read a spread of rows rather than all hitting
    # row 0: an all-identical gather index pattern measured ~15% slower.
    spread = ((np.arange(P)[:, None] * NTILES + np.arange(NB)[None, :])
              % CHUNK).astype(np.int16)
    src16 = np.broadcast_to(spread, (NCORES, P, NB)).copy()
    srcg = np.zeros((NCORES, P, NB), np.int32)   # emulator only
    offb = np.full((NCORES, P, NB), 255.0, np.float32)

    order = np.argsort(key, kind="stable")
    key_s = key[order]
    cg = key_s // (NTILES * NCHUNK)
    wg = (key_s // NCHUNK) % NTILES
    chg = key_s % NCHUNK
    grp_start = np.concatenate([[0], np.cumsum(cnt.ravel())[:-1]])
    j = np.arange(key_s.size) - grp_start[key_s]
    b = slot_base[wg, chg] + j // P
    p = j % P
    src16[cg, p, b] = lsrc[order].astype(np.int16)
    srcg[cg, p, b] = gsrc[order].astype(np.int32)
    offb[cg, p, b] = off[order].astype(np.float32)

    deg = np.bincount(col, minlength=N_NODES).astype(np.float32) + 1.0
    dinv = 1.0 / np.sqrt(deg)

    maxsegb = max(b1 - b0 for _, b0, b1 in segs)
    return dict(NB=NB, win_of=win_of, chunk_of=chunk_of, first=first,
                last=last, segs=segs, maxsegb=maxsegb,
                src16=src16, srcg=srcg, offb=offb, dinv=dinv, lslot=lslot)


def build_core_inputs(inputs, sched):
    """Per-core named arrays (consumed by emulate + pack_device_maps)."""
    x = np.asarray(inputs["x"], np.float32)
    batch = np.asarray(inputs["batch"], np.int32)
    dinv = sched["dinv"]

    common = {
        "W_in": np.asarray(inputs["W_in"], np.float32),
        "W_mid": np.asarray(inputs["W_mid"], np.float32),
        "W_out": np.asarray(inputs["W_out"], np.float32),
        "W1": np.asarray(inputs["W1"], np.float32),
        "W2": np.asarray(inputs["W2"], np.float32),
        "identity": np.eye(P, dtype=np.float32),
        "iota_bf": np.tile(np.arange(P, dtype=np.float32), (P, 1)).astype(BF),
        "iota_f32": np.tile(np.arange(P, dtype=np.float32), (P, 1)),
        "ones_col": np.ones((P, 1), np.float32),
        "epsb": np.full((P, 1), EPS, np.float32),
        "b2_bc": np.full((P, 1), float(np.asarray(inputs["b2"])[0]), np.float32),
        "b1_bc": np.tile(np.asarray(inputs["b1"], np.float32), (P, 1)),
    }
    for nm in ("in", "mid", "out"):
        common[f"b_{nm}_bc"] = np.tile(np.asarray(inputs[f"b_{nm}"], np.float32),
                                       (P, 1))
        common[f"g_{nm}_bc"] = np.tile(np.asarray(inputs[f"g_{nm}"], np.float32),
                                       (P, 1))
        common[f"be_{nm}_bc"] = np.tile(np.asarray(inputs[f"be_{nm}"], np.float32),
                                        (P, 1))

    maps = []
    for k in range(NCORES):
        lo, hi = k * OWN, (k + 1) * OWN
        ls = sched["lslot"][k]
        xs = np.zeros((R, HID), np.float32)
        xs[ls] = x[lo:hi]
        tmp = np.zeros(R, np.float32)
        tmp[ls] = dinv[lo:hi]
        dv = np.ascontiguousarray(tmp.reshape(NTILES, P).T)
        dv2 = np.ascontiguousarray((tmp * tmp).reshape(NTILES, P).T)
        tmp = np.full(R, 255.0, np.float32)
        tmp[ls] = batch[lo:hi].astype(np.float32)
        bb = np.ascontiguousarray(tmp.reshape(NTILES, P).T)
        m = dict(common)
        m.update({
            "xs": xs,
            "src16": sched["src16"][k],
            "srcg": sched["srcg"][k],
            "offb": sched["offb"][k].astype(BF),
            "dinvs": dv,
            "dinv2s": dv2,
            "batchb": bb,
        })
        maps.append(m)
    return maps


def pack_device_maps(maps, sched):
    """Pack named arrays into device in_maps."""
    NB = sched["NB"]
    dev = []
    for m in maps:
        cb = np.zeros((P, CB_COLS), np.float32)
        for n, w in _CB_LAYOUT:
            o = CB_OFF[n]
            a = np.asarray(m[n], np.float32)
            if n == "W2":
                cb[:FC_DIM, o:o + 1] = a
            else:
                cb[:, o:o + w] = a
        # idx16: per segment, wrap idxs (j = batch*128 + p) into 16
        # partitions, col j//16; replicate to all 8 gpsimd core groups.
        idx16 = np.zeros((P, NB * 8), np.int16)
        for c, b0, b1 in sched["segs"]:
            jarr = m["src16"][:, b0:b1].T.reshape(-1)        # [n] batch-major
            blk = jarr.reshape(-1, 16).T                     # [16, n/16]
            idx16[:, b0 * 8:b1 * 8] = np.tile(blk, (8, 1))
        dev.append({
            "xs": m["xs"].astype(BF),
            "srcidx": idx16,
            "offb": m["offb"],
            "iota_bf": m["iota_bf"],
            "cblob": cb,
        })
    return dev


# ------------------------------------------------------------ numpy emulator
def emulate(inputs, sched, maps):
    """Emulation of the device numerics (bf16 activations/weights)."""
    NB = sched["NB"]
    win_of = sched["win_of"]
    layers = (["in"] + ["mid"] * 6 + ["out"])

    xs = [m["xs"].astype(BF) for m in maps]
    for li, nm in enumerate(layers):
        W = maps[0][f"W_{nm}"].astype(BF).astype(np.float32)
        b = maps[0][f"b_{nm}_bc"][0]
        g = maps[0][f"g_{nm}_bc"][0]
        be = maps[0][f"be_{nm}_bc"][0]
        import ml_dtypes as _md
        F8 = _md.float8_e4m3
        hp = []
        hp2s = []
        for k in range(NCORES):
            h = xs[k].astype(np.float32) @ W
            dv = maps[k]["dinvs"].T.reshape(-1, 1)
            dv2 = maps[k]["dinv2s"].T.reshape(-1, 1)
            hp.append((h * dv).astype(F8))
            hp2s.append((hp[-1].astype(np.float32) * dv).astype(F8))
        h_full = np.concatenate(hp, 0).astype(BF).astype(np.float32)
        nxt = []
        for k in range(NCORES):
            m = maps[k]
            out = np.zeros((R, HID), np.float32)
            for bidx in range(NB):
                w = win_of[bidx]
                G = h_full[m["srcg"][:, bidx]]
                offv = m["offb"][:, bidx].astype(np.float32)
                Pm = (offv[:, None] == np.arange(P)).astype(np.float32)
                out[w * P:(w + 1) * P] += Pm.T @ G
            dv = m["dinvs"].T.reshape(-1, 1)
            z = (out * dv + hp2s[k].astype(np.float32) + b).astype(BF)
            z = z.astype(np.float32)
            mu = z.mean(1, keepdims=True)
            var = ((z - mu) ** 2).mean(1, keepdims=True)
            y = ((z - mu) / np.sqrt(var + EPS) * g + be).astype(BF)
            y = np.maximum(y.astype(np.float32), 0.0)
            if nm == "mid" and li % 2 == 0:
                y = (y.astype(BF).astype(np.float32) +
                     xs[k].astype(np.float32)).astype(BF)
            else:
                y = y.astype(BF)
            nxt.append(y)
        xs = nxt

    sums = np.zeros((NUM_GRAPHS, HID + 1), np.float32)
    for k in range(NCORES):
        bb = maps[k]["batchb"].astype(np.float32).T.reshape(-1)
        valid = bb < NUM_GRAPHS
        idx = bb[valid].astype(np.int64)
        np.add.at(sums[:, :HID], idx, xs[k].astype(np.float32)[valid])
        np.add.at(sums[:, HID], idx, 1.0)
    pooled = sums[:, :HID] / np.maximum(sums[:, HID:], 1.0)
    z = np.maximum(pooled @ maps[0]["W1"] + maps[0]["b1_bc"][0], 0.0)
    o = z @ maps[0]["W2"] + maps[0]["b2_bc"][0, 0]
    return 1.0 / (1.0 + np.exp(-o))


# ------------------------------------------------------------- bass program
def build_nc(sched, taps=()):
    import concourse.bass as bass
    import concourse.bacc as bacc
    import concourse.tile as tile
    from concourse import mybir

    NB = sched["NB"]
    win_of, first, last = sched["win_of"], sched["first"], sched["last"]
    segs, maxsegb = sched["segs"], sched["maxsegb"]
    f32 = mybir.dt.float32
    bf16 = mybir.dt.bfloat16
    f8 = mybir.dt.float8e4
    i16 = mybir.dt.int16
    AF = mybir.ActivationFunctionType
    OP = mybir.AluOpType

    nc = bacc.Bacc("TRN2", target_bir_lowering=False, debug=False,
                   num_devices=NCORES, num_swdge_queues=4)

    xs_d = nc.dram_tensor("xs", [R, HID], bf16, kind="ExternalInput")
    src_d = nc.dram_tensor("srcidx", [P, NB * 8], i16, kind="ExternalInput")
    off_d = nc.dram_tensor("offb", [P, NB], bf16, kind="ExternalInput")
    cb_d = nc.dram_tensor("cblob", [P, CB_COLS], f32, kind="ExternalInput")
    iobf_d = nc.dram_tensor("iota_bf", [P, P], bf16, kind="ExternalInput")
    out_d = nc.dram_tensor("out", [NUM_GRAPHS, 1], f32, kind="ExternalOutput")
    tap_d = {}
    for tn, shp, dt in [("hfull0", [GPAD, HID], mybir.dt.float8e4),
                        ("x1", [P, NTILES, P], bf16),
                        ("G0", [P, sched["maxsegb"] * P], bf16),
                        ("P0", [P, sched["maxsegb"] * P], bf16)]:
        if tn in taps:
            tap_d[tn] = nc.dram_tensor(tn, shp, dt, kind="ExternalOutput")

    layers = (["in"] + ["mid"] * 6 + ["out"])

    with tile.TileContext(nc) as tc:
        with (
            tc.tile_pool(name="singles", bufs=1) as singles,
            tc.tile_pool(name="xab", bufs=1) as xab,
            tc.tile_pool(name="sbA", bufs=3) as sbA,
            tc.tile_pool(name="hppool", bufs=1) as hppool,
            tc.tile_pool(name="zstrips", bufs=1) as zstrips,
            tc.tile_pool(name="statp", bufs=1) as statp,
            tc.tile_pool(name="ipool", bufs=1) as ipool,
            tc.tile_pool(name="gpool", bufs=4) as gpool,
            tc.tile_pool(name="ppool", bufs=3) as ppool,
            tc.tile_pool(name="zpool", bufs=3) as zpool,
            tc.tile_pool(name="spool", bufs=4) as spool,
            tc.tile_pool(name="psA", bufs=3, space="PSUM") as psA,
            tc.tile_pool(name="psW", bufs=4, space="PSUM") as psW,
            tc.tile_pool(name="psP", bufs=1, space="PSUM") as psP,
            tc.tile_pool(name="dram", bufs=2, space="DRAM") as dram,
            tc.tile_pool(name="dram1", bufs=1, space="DRAM") as dram1,
        ):
            # ---- constants
            cblob = singles.tile([P, CB_COLS], f32, name="cblob")
            nc.sync.dma_start(cblob[:], cb_d[:, :])
            consts = {}
            for name, o, w in _cb_slices():
                consts[name] = cblob[:, o:o + w]
            consts["W2"] = cblob[0:FC_DIM, CB_OFF["W2"]:CB_OFF["W2"] + 1]
            iota_sb = singles.tile([P, P], bf16, name="iota_sb")
            nc.sync.dma_start(iota_sb[:], iobf_d[:, :])
            off_sb = singles.tile([P, NB], bf16, name="off_sb")
            nc.sync.dma_start(off_sb[:], off_d[:, :])
            dinv_sb = consts["dinvs"]
            dinv2_sb = consts["dinv2s"]
            bat_sb = consts["batchb"]

            # bf16 copies of matmul weights / identity
            wbf = {}
            for nm in ("in", "mid", "out"):
                t = singles.tile([P, P], bf16, name=f"Wbf_{nm}")
                nc.vector.tensor_copy(t[:], consts[f"W_{nm}"])
                wbf[nm] = t
            ident_bf = singles.tile([P, P], bf16, name="ident_bf")
            nc.vector.tensor_copy(ident_bf[:], consts["identity"])

            xa = xab.tile([P, NTILES, P], bf16, name="xa")
            xb = xab.tile([P, NTILES, P], bf16, name="xb")
            nc.sync.dma_start(
                xa[:], xs_d.rearrange("(t p) f -> p t f", p=P))

            onesbf = singles.tile([P, 1], bf16, name="onesbf")
            nc.vector.tensor_copy(onesbf[:], consts["ones_col"][:])

            def dense_phase(xcur, nm):
                """h' = dinv*(x @ W) -> SBUF strip -> one [128,12544] store.

                The self-loop term dinv^2 * (x @ W) is derived from this
                strip by the strip post (one big DVE multiply)."""
                h_own = dram.tile([P, NTILES * P], f8, name="h_own")
                hp_strip = hppool.tile([P, NTILES, P], f8, name="hp_strip")
                W = wbf[nm]
                for t in range(NTILES):
                    xT_ps = psA.tile([P, P], bf16, name="xT_ps", tag="psa")
                    nc.tensor.transpose(xT_ps[:], xcur[:, t, :], ident_bf[:])
                    xT_sb = sbA.tile([P, P], bf16, name="xT_sb")
                    nc.vector.tensor_copy(xT_sb[:], xT_ps[:])
                    h_ps = psA.tile([P, P], f32, name="h_ps", tag="psa")
                    nc.tensor.matmul(h_ps[:], lhsT=xT_sb[:], rhs=W[:],
                                     start=True, stop=True)
                    nc.scalar.activation(hp_strip[:, t, :], h_ps[:], AF.Copy,
                                         scale=dinv_sb[:, t:t + 1])
                nc.sync.dma_start(h_own[:, :], hp_strip[:])
                return h_own, hp_strip

            def scatter_phase(h_full, Tbf, hp_strip, xcur, xnxt, nm,
                              residual, idx_strip, zstrip, tap=False,
                              half_cb=None):
                accs = {}
                HALF = (NTILES // 2 // WB) * WB   # window-block boundary
                done = [0]
                for si, (c, b0, b1) in enumerate(segs):
                    nb = b1 - b0
                    isl = idx_strip[:, b0 * 8:b1 * 8]
                    # gather base address is a 64-bit immediate from the AP,
                    # so slicing the big bf16 tensor at the chunk offset is
                    # fine; only the int16 index must stay < 32768.
                    src_ap = Tbf[c * CHUNK:(c + 1) * CHUNK, :]

                    Gt = gpool.tile([P, maxsegb * P], bf16, name="Gt")
                    nc.gpsimd.dma_gather(
                        out_ap=Gt[:, :nb * P].rearrange(
                            "p (b f) -> p b f", b=nb),
                        in_ap=src_ap,
                        idxs_ap=isl,
                        num_idxs=nb * P,
                        num_idxs_reg=nb * P,
                        elem_size=P,
                        single_packet=False,
                        queue_num=si % 4,
                    )
                    if tap and si == 0 and "G0" in tap_d:
                        nc.sync.dma_start(tap_d["G0"][:, :nb * P],
                                          Gt[:, :nb * P])
                    Pt = ppool.tile([P, maxsegb * P], bf16, name="Pt")
                    o = off_sb[:, b0:b1]
                    off_b = bass.AP(tensor=o.tensor, offset=o.offset,
                                    ap=list(o.ap) + [[0, P]])
                    i0 = iota_sb[:, :]
                    iota_b = bass.AP(tensor=i0.tensor, offset=i0.offset,
                                     ap=[i0.ap[0], [0, nb], [1, P]])
                    nc.vector.tensor_tensor(
                        out=Pt[:, :nb * P].rearrange("p (b f) -> p b f", b=nb),
                        in0=off_b, in1=iota_b, op=OP.is_equal)
                    if tap and si == 0 and "P0" in tap_d:
                        nc.sync.dma_start(tap_d["P0"][:, :nb * P],
                                          Pt[:, :nb * P])
                    for bi in range(b0, b1):
                        w = int(win_of[bi])
                        s = bi - b0
                        if first[bi]:
                            accs[w] = psW.tile([P, P], f32, name="acc")
                        acc = accs[w]
                        nc.tensor.matmul(
                            acc[:], lhsT=Pt[:, s * P:(s + 1) * P],
                            rhs=Gt[:, s * P:(s + 1) * P],
                            start=bool(first[bi]), stop=bool(last[bi]))
                        if last[bi]:
                            del accs[w]
                            # z[w] = acc * dinv  (self/bias/LN done on the
                            # whole strip afterwards with big ops)
                            nc.scalar.activation(zstrip[:, w, :], acc[:],
                                                 AF.Copy,
                                                 scale=dinv_sb[:, w:w + 1])
                            done[0] += 1
                            if done[0] == HALF and half_cb is not None:
                                half_cb(0, HALF)
                if half_cb is not None:
                    half_cb(HALF, NTILES)

            def strip_post(zstrip, hp_strip, xcur, xnxt, nm, residual,
                           t0, t1):
                """Self-term + bias + LayerNorm + ReLU on tiles [t0, t1)."""
                nt = t1 - t0
                b_bc = consts[f"b_{nm}_bc"]
                g_bc = consts[f"g_{nm}_bc"]
                be_bc = consts[f"be_{nm}_bc"]

                def bc_t(const_pp):
                    # [P, P] feature constant -> [p, {0,nt}, f]
                    a = const_pp[:, :]
                    return bass.AP(tensor=a.tensor, offset=a.offset,
                                   ap=[a.ap[0], [0, nt], a.ap[1]])

                def bc_f(strip2d):
                    # [P, NTILES]-shaped AP -> [p, t, {0,P}]
                    a = strip2d
                    return bass.AP(tensor=a.tensor, offset=a.offset,
                                   ap=[a.ap[0], a.ap[1], [0, P]])

                # self term: hp * dinv (hp already carries one dinv factor)
                zs = zstrip[:, t0:t1, :]
                hp2t = hppool.tile([P, NTILES, P], f8, name="hp2t",
                                   tag="hp2t")
                dv = dinv_sb[:, t0:t1]
                nc.vector.tensor_tensor(
                    out=hp2t[:, t0:t1, :], in0=hp_strip[:, t0:t1, :],
                    in1=bass.AP(tensor=dv.tensor, offset=dv.offset,
                                ap=list(dv.ap) + [[0, P]]),
                    op=OP.mult)
                nc.vector.tensor_add(zs, zs, hp2t[:, t0:t1, :])
                nc.vector.tensor_tensor(out=zs, in0=zs,
                                        in1=bc_t(b_bc), op=OP.add)
                stats6 = statp.tile([P, NTILES, 6], f32, name="stats6",
                                    tag="st6")
                mv = statp.tile([P, NTILES, 2], f32, name="mvs", tag="mvs")
                for t in range(t0, t1):
                    nc.vector.bn_stats(stats6[:, t, :], zstrip[:, t, :])
                    nc.vector.bn_aggr(mv[:, t, :], stats6[:, t, :])
                mu2 = mv[:, t0:t1, 0:1]
                var2 = mv[:, t0:t1, 1:2]
                sd = statp.tile([P, NTILES], f32, name="sds", tag="sds")
                nc.scalar.activation(
                    sd[:, t0:t1],
                    bass.AP(tensor=var2.tensor, offset=var2.offset,
                            ap=[var2.ap[0], var2.ap[1]]),
                    AF.Sqrt, bias=consts["epsb"][:])
                rstd = statp.tile([P, NTILES], f32, name="rstds", tag="rst")
                nc.vector.reciprocal(rstd[:, t0:t1], sd[:, t0:t1])
                nc.vector.tensor_tensor(
                    out=zs, in0=zs,
                    in1=bass.AP(tensor=mu2.tensor, offset=mu2.offset,
                                ap=[mu2.ap[0], mu2.ap[1], [0, P]]),
                    op=OP.subtract)
                rs = rstd[:, t0:t1]
                nc.vector.tensor_tensor(out=zs, in0=zs,
                                        in1=bc_f(rs), op=OP.mult)
                nc.vector.tensor_tensor(out=zs, in0=zs,
                                        in1=bc_t(g_bc), op=OP.mult)
                nc.vector.tensor_tensor(out=zs, in0=zs,
                                        in1=bc_t(be_bc), op=OP.add)
                if residual:
                    nc.vector.tensor_scalar_max(out=zs, in0=zs, scalar1=0.0)
                    nc.vector.tensor_add(xnxt[:, t0:t1, :], zs,
                                         xcur[:, t0:t1, :])
                else:
                    nc.scalar.activation(xnxt[:, t0:t1, :], zs, AF.Relu)

            cur, nxt = xa, xb
            idx_strip = ipool.tile([P, NB * 8], i16, name="idx_strip")
            nc.sync.dma_start(idx_strip[:], src_d[:, :])
            for li, nm in enumerate(layers):
                h_own, hp_strip = dense_phase(cur, nm)
                h_full = dram.tile([P * NCORES, NTILES * P], f8,
                                   addr_space="Shared", name="h_full")
                nc.gpsimd.collective_compute(
                    "AllGather", OP.bypass,
                    replica_groups=[list(range(NCORES))],
                    ins=[h_own[:, :].opt()], outs=[h_full[:, :].opt()])
                # rank strips in [p, t, f] order concatenate to exactly the
                # [100352, 128] chunk-stacked row layout; the f8 -> bf16
                # upconvert is a flat converting DMA per chunk (per-chunk so
                # gathers on chunk c start as soon as its convert lands).
                Tbf_t = dram.tile([GPAD, HID], bf16, name="Tbf")
                for c in range(NCHUNK):
                    nc.gpsimd.dma_start(
                        Tbf_t[c * CHUNK:(c + 1) * CHUNK, :].rearrange(
                            "(q r) f -> q (r f)", q=2 * P),
                        h_full[c * 2 * P:(c + 1) * 2 * P, :])
                Tbf = Tbf_t[:, :]
                residual = (nm == "mid" and li % 2 == 0)
                zstrip = zstrips.tile([P, NTILES, P], bf16, name="zstrip",
                                      tag="zs")
                scatter_phase(h_full, Tbf, hp_strip, cur, nxt, nm,
                              residual, idx_strip, zstrip, tap=(li == 0),
                              half_cb=lambda t0, t1: strip_post(
                                  zstrip, hp_strip, cur, nxt, nm, residual,
                                  t0, t1))
                if li == 0 and "x1" in tap_d:
                    nc.sync.dma_start(tap_d["x1"][:, :, :], nxt[:])
                cur, nxt = nxt, cur

            # ---- pooling
            # one-hot strip built here so its buffer (zstrips pool) is free
            # during the layers: Bstrip[p, t, g] = (batch[p,t] == g)
            Bstrip = zstrips.tile([P, NTILES, P], bf16, name="Bstrip",
                                  tag="zs")
            _bat = bat_sb[:, :]
            bat_b = bass.AP(tensor=_bat.tensor, offset=_bat.offset,
                            ap=list(_bat.ap) + [[0, P]])
            _io = consts["iota_f32"][:, :]
            iota_rep = bass.AP(tensor=_io.tensor, offset=_io.offset,
                               ap=[_io.ap[0], [0, NTILES], [1, P]])
            nc.vector.tensor_tensor(out=Bstrip[:], in0=bat_b, in1=iota_rep,
                                    op=OP.is_equal)
            pool_ps = psP.tile([P, HID + 1], f32, name="pool_ps")
            for t in range(NTILES):
                nc.tensor.matmul(pool_ps[:, :HID], lhsT=Bstrip[:, t, :],
                                 rhs=cur[:, t, :],
                                 start=(t == 0), stop=(t == NTILES - 1),
                                 skip_group_check=True)
                nc.tensor.matmul(pool_ps[:, HID:HID + 1], lhsT=Bstrip[:, t, :],
                                 rhs=onesbf[:],
                                 start=(t == 0), stop=(t == NTILES - 1),
                                 skip_group_check=True)
            pool_sb = zpool.tile([P, HID + 1], f32, name="pool_sb")
            nc.vector.tensor_copy(pool_sb[:], pool_ps[:])
            pool_in = dram1.tile([P, HID + 1], f32, name="pool_in")
            pool_out = dram1.tile([P, HID + 1], f32, addr_space="Shared",
                                  name="pool_out")
            nc.sync.dma_start(pool_in[:, :], pool_sb[:])
            nc.gpsimd.collective_compute(
                "AllReduce", OP.add, replica_groups=[list(range(NCORES))],
                ins=[pool_in[:, :].opt()], outs=[pool_out[:, :].opt()])
            pooled = zpool.tile([P, HID + 1], f32, name="pooled")
            nc.sync.dma_start(pooled[:], pool_out[:, :])

            cnt = spool.tile([P, 1], f32, name="cnt")
            nc.vector.tensor_copy(cnt[:], pooled[:, HID:HID + 1])
            nc.vector.tensor_scalar_max(out=cnt[:], in0=cnt[:], scalar1=1.0)
            crec = spool.tile([P, 1], f32, name="crec")
            nc.vector.reciprocal(crec[:], cnt[:])
            pm = zpool.tile([P, HID], f32, name="pm")
            nc.vector.tensor_scalar_mul(out=pm[:], in0=pooled[:, :HID],
                                        scalar1=crec[:])
            pmT_ps = psA.tile([P, P], f32, name="pmT_ps", tag="psa")
            nc.tensor.transpose(pmT_ps[:], pm[:], consts["identity"])
            pmT = sbA.tile([P, P], f32, name="pmT")
            nc.vector.tensor_copy(pmT[:], pmT_ps[:])
            z1_ps = psA.tile([P, FC_DIM], f32, name="z1_ps", tag="psa")
            nc.tensor.matmul(z1_ps[:], lhsT=pmT[:], rhs=consts["W1"],
                             start=True, stop=True)
            z1 = zpool.tile([P, FC_DIM], f32, name="z1")
            nc.vector.tensor_add(z1[:], z1_ps[:], consts["b1_bc"])
            nc.scalar.activation(z1[:], z1[:], AF.Relu)
            z1T_ps = psA.tile([FC_DIM, P], f32, name="z1T_ps", tag="psa")
            nc.tensor.transpose(z1T_ps[:], z1[:], consts["identity"])
            z1T = sbA.tile([FC_DIM, P], f32, name="z1T")
            nc.vector.tensor_copy(z1T[:], z1T_ps[:])
            o_ps = psA.tile([P, 1], f32, name="o_ps", tag="psa")
            nc.tensor.matmul(o_ps[:], lhsT=z1T[:], rhs=consts["W2"],
                             start=True, stop=True)
            o_sb = spool.tile([P, 1], f32, name="o_sb")
            nc.scalar.activation(o_sb[:], o_ps[:], AF.Sigmoid,
                                 bias=consts["b2_bc"][:])
            nc.sync.dma_start(out_d[:, :], o_sb[:])

    nc.compile()
    return nc


# ----------------------------------------------------------------- entry
_CACHE = {}


def kernel(**inputs):
    from concourse import bass_utils

    edge_index = np.asarray(inputs["edge_index"])
    sched = build_schedule(edge_index)
    maps = build_core_inputs(inputs, sched)

    key = sched["NB"]
    if key not in _CACHE:
        _CACHE[key] = build_nc(sched)
    nc = _CACHE[key]

    res = bass_utils.run_bass_kernel_spmd(
        nc, pack_device_maps(maps, sched), core_ids=list(range(NCORES)))
    return np.asarray(res.results[0]["out"], np.float32)

